# revision 1
# baseline (speedup 1.0000x reference)
"""BERT multi-head attention forward on 8 Trainium2 NeuronCores.

Sharding: tensor-parallel over heads (16 heads -> 2 per core) for the QKV
projection and attention; then a per-batch AllToAll redistributes the
attention outputs token-wise so each core computes the output projection for
its own 512-token slice (no AllReduce needed).

Layouts (per core):
  xT      [E=1024, T=4096]   x transposed (embed on partitions)
  wqkvT   [1024, 384]        this core's Wqkv rows (qA qB kA kB vA vB), transposed
  qkvT    [384, 4096] SBUF   j rows: q(128) k(128) v(128); each 128 = headA(64)+headB(64)
  scoresT [keys, queries]    computed per 128-key x 512-query block, exp'd on ACT
  V_aug   [128 keys, 65]     V natural layout + ones column (fused sum-of-exp row)
  out_aug [65, 512] PSUM     rows 0-63 = unnormalized attn out (d x q), row 64 = sumexp
  concatT [128, 4096] SBUF   this core's 2 heads' channels x all tokens
  A2A     per batch: blocks of [128 ch, 256 tok] -> core j gets all 1024 channels
                     for its 256 tokens of that batch
  outT    [1024 e_out, 512]  output projection result, tokens of this core

All matmuls run as float32r (TF32-class, ~1.5e-4 rel err, 4x faster than
plain fp32 on the PE).
"""

import numpy as np
from concourse import bacc, tile, bass_utils, mybir

F32 = mybir.dt.float32
F32R = mybir.dt.float32r
AF = mybir.ActivationFunctionType

B, S, E, H, D = 2, 2048, 1024, 16, 64
T = B * S                  # 4096 tokens
N_CORES = 8
HPC = H // N_CORES         # 2 heads per core
TC = 512                   # t-chunk for QKV projection
QC = 512                   # query chunk in attention
KT_S = S // 128            # 16 key tiles per batch
TPB = T // B // N_CORES    # 256 tokens per core per batch (A2A block)

PV_MODE = "f32r"   # "f32r" (exp->f32, DVE round to f32r, ~4.5e-4) or "bf16" (~2.6e-3, ~20% faster)
BF16 = mybir.dt.bfloat16
_CACHE = {}


def _build(k_rep=1, phases=(1, 2, 3, 4, 5)):
    key = (k_rep, tuple(phases), PV_MODE)
    if key in _CACHE:
        return _CACHE[key]
    nc = bacc.Bacc("TRN2", target_bir_lowering=False, debug=False, num_devices=N_CORES)

    xT = nc.dram_tensor("xT", [E, T], F32, kind="ExternalInput").ap()
    wqkvT = nc.dram_tensor("wqkvT", [E, 3 * 128], F32, kind="ExternalInput").ap()
    bqkv_d = nc.dram_tensor("bqkv_sb", [128, 3], F32, kind="ExternalInput").ap()
    woutT = nc.dram_tensor("woutT", [E, E], F32, kind="ExternalInput").ap()
    bout_d = nc.dram_tensor("bout_sb", [128, 8], F32, kind="ExternalInput").ap()
    abias_d = nc.dram_tensor("abias_sb", [128, B * KT_S], F32, kind="ExternalInput").ap()
    ident_d = nc.dram_tensor("ident", [128, 64], F32, kind="ExternalInput").ap()
    chain_d = nc.dram_tensor("chain", [1, 128], F32, kind="ExternalInput").ap()

    outT_d = nc.dram_tensor("outT", [E, 2 * TPB], F32, kind="ExternalOutput").ap()
    chout_d = nc.dram_tensor("chain_out", [1, 128], F32, kind="ExternalOutput").ap()

    with tile.TileContext(nc) as tc:
        with tc.tile_pool(name="sb", bufs=1) as sb, \
             tc.tile_pool(name="ps_attn", bufs=1, space="PSUM") as ps_attn, \
             tc.tile_pool(name="dram", bufs=1, space="DRAM") as dram:

            # chain passthrough (timing harness hook; negligible cost)
            ch_sb = sb.tile([1, 128], F32)
            nc.sync.dma_start(ch_sb[:], chain_d[:])
            nc.vector.tensor_copy(ch_sb[:], ch_sb[:])
            nc.sync.dma_start(chout_d[:], ch_sb[:])

            # ---- constants ----
            bqkv_sb = sb.tile([128, 3], F32)
            bout_sb = sb.tile([128, 8], F32)
            abias_sb = sb.tile([128, B * KT_S], F32)
            ident_sb = sb.tile([128, 64], F32R)
            ones_sb = sb.tile([1, 64], F32R)
            nc.sync.dma_start(bqkv_sb[:], bqkv_d[:])
            nc.sync.dma_start(bout_sb[:], bout_d[:])
            nc.sync.dma_start(abias_sb[:], abias_d[:])
            nc.sync.dma_start(ident_sb[:], ident_d[:].bitcast(F32R))
            nc.vector.memset(ones_sb[:].bitcast(F32), 1.0)

            # ---- weights ----
            wq_sb = [sb.tile([128, 3 * 128], F32R, name=f"wq_{e}") for e in range(8)]
            for e in range(8):
                nc.sync.dma_start(wq_sb[e][:], wqkvT[e * 128:(e + 1) * 128, :].bitcast(F32R))
            wo_sb = [sb.tile([128, E], F32R, name=f"wo_{e}") for e in range(8)]
            for e in range(8):
                nc.sync.dma_start(wo_sb[e][:], woutT[e * 128:(e + 1) * 128, :].bitcast(F32R))

            for _rep in range(k_rep):
                # ---- phase 1: QKV projection (d-major output) ----
                qkvT = [sb.tile([128, T], F32R, name=f"qkvT_{j}") for j in range(3)]
                concatT = sb.tile([128, T], F32)
                if 1 not in phases and (2 in phases or 3 in phases):
                    for j in range(3):
                        nc.vector.memset(qkvT[j][:].bitcast(F32), 0.01)
                if 3 not in phases and (4 in phases or 5 not in phases):
                    nc.vector.memset(concatT[:], 0.25)
                for i in range(T // TC) if 1 in phases else []:
                    xt = [sb.tile([128, TC], F32R, name="xt", tag=f"xt{e}", bufs=2)
                          for e in range(8)]
                    for e in range(8):
                        nc.sync.dma_start(
                            xt[e][:], xT[e * 128:(e + 1) * 128, i * TC:(i + 1) * TC].bitcast(F32R))
                    for j in range(3):
                        acc = ps_attn.tile([128, TC], F32, name="acc", tag="sc", bufs=2)
                        for e in range(8):
                            nc.tensor.matmul(acc[:], wq_sb[e][:, j * 128:(j + 1) * 128],
                                             xt[e][:], start=(e == 0), stop=(e == 7))
                        with nc.allow_low_precision(reason="fp32r rounding for PE"):
                            nc.scalar.activation(qkvT[j][:, i * TC:(i + 1) * TC], acc[:],
                                                 AF.Identity, bias=bqkv_sb[:, j:j + 1])

                # ---- phase 2: V natural layout via PE transposes, + ones column ----
                # vaug[h][b][kt]: [128 keys, 65]
                vaug = {}
                for b in range(B) if (2 in phases or 3 in phases) else []:
                    for h in range(HPC):
                        for kt in range(KT_S):
                            v = sb.tile([128, 65], F32R if PV_MODE == "f32r" else BF16,
                                        name=f"vaug_{h}_{b}_{kt}")
                            if 2 in phases:
                                tp = ps_attn.tile([128, 64], F32R, name="tp", tag="sc", bufs=2)
                                nc.tensor.transpose(
                                    tp[:],
                                    qkvT[2][64 * h:64 * h + 64,
                                            b * S + kt * 128: b * S + (kt + 1) * 128],
                                    ident_sb[64 * h:64 * h + 64, 0:64])
                                with nc.allow_low_precision(reason="fp32r rounding for PE"):
                                    nc.scalar.activation(v[:, 0:64], tp[:], AF.Copy)
                                if PV_MODE == "f32r":
                                    nc.vector.memset(v[:, 64:65].bitcast(F32), 1.0)
                                else:
                                    nc.vector.memset(v[:, 64:65], 1.0)
                            else:
                                if PV_MODE == "f32r":
                                    nc.vector.memset(v[:].bitcast(F32), 0.5)
                                else:
                                    nc.vector.memset(v[:], 0.5)
                            vaug[h, b, kt] = v

                # ---- phase 3: attention (qc pairs; same-lhsT matmuls adjacent) ----
                for b in range(B) if 3 in phases else []:
                    for qcp in range(S // QC // 2):
                        q0s = [b * S + (2 * qcp + q) * QC for q in range(2)]
                        oaug = {(h, q): ps_attn.tile([65, QC], F32, name=f"oaug{h}{q}",
                                                     tag=f"oaug{h}{q}")
                                for h in range(HPC) for q in range(2)}
                        for kt in range(KT_S):
                            k0 = b * S + kt * 128
                            exs = {}
                            for h in range(HPC):
                                scp = ps_attn.tile([128, 2 * QC], F32, name="sc",
                                                   tag="sc", bufs=2)
                                for q in range(2):
                                    nc.tensor.matmul(
                                        scp[:, q * QC:(q + 1) * QC],
                                        qkvT[1][64 * h:64 * h + 64, k0:k0 + 128],
                                        qkvT[0][64 * h:64 * h + 64, q0s[q]:q0s[q] + QC],
                                        start=True, stop=True)
                                bias_ap = abias_sb[:, b * KT_S + kt: b * KT_S + kt + 1]
                                if PV_MODE == "f32r":
                                    exf = sb.tile([128, 2 * QC], F32, name="exf",
                                                  tag="exf", bufs=2)
                                    nc.scalar.activation(exf[:], scp[:], AF.Exp,
                                                         scale=0.125, bias=bias_ap)
                                    ex = sb.tile([128, 2 * QC], F32R, name="ex",
                                                 tag="ex", bufs=3)
                                    with nc.allow_low_precision(reason="fp32r rounding for PE"):
                                        nc.vector.tensor_copy(ex[:], exf[:])
                                else:
                                    ex = sb.tile([128, 2 * QC], BF16, name="ex",
                                                 tag="ex", bufs=3)
                                    with nc.allow_low_precision(reason="bf16 attn weights"):
                                        nc.scalar.activation(ex[:], scp[:], AF.Exp,
                                                             scale=0.125, bias=bias_ap)
                                exs[h] = ex
                            for h in range(HPC):
                                for q in range(2):
                                    nc.tensor.matmul(oaug[h, q][:], vaug[h, b, kt][:],
                                                     exs[h][:, q * QC:(q + 1) * QC],
                                                     start=(kt == 0), stop=(kt == KT_S - 1))
                        for h in range(HPC):
                            for q in range(2):
                                sumrow = sb.tile([1, QC], F32R, name="sumrow", tag="sumrow", bufs=2)
                                with nc.allow_low_precision(reason="fp32r rounding for PE"):
                                    nc.vector.tensor_copy(sumrow[:], oaug[h, q][64:65, :])
                                rep = ps_attn.tile([64, QC], F32, name="rep", tag="sc", bufs=2)
                                nc.tensor.matmul(rep[:], ones_sb[:], sumrow[:],
                                                 start=True, stop=True)
                                reprecip = sb.tile([64, QC], F32, name="reprecip",
                                                   tag="reprecip", bufs=2)
                                nc.vector.reciprocal(reprecip[:], rep[:])
                                nc.vector.tensor_mul(
                                    concatT[64 * h:64 * h + 64, q0s[q]:q0s[q] + QC],
                                    oaug[h, q][0:64, :], reprecip[:])

                # ---- phase 4: per-batch AllToAll (blocks of [128 ch, 256 tok]) ----
                a2a_out = []
                for b in range(B) if 4 in phases else []:
                    a2a_in_b = dram.tile([N_CORES * 128, TPB], F32, name=f"a2a_in_{b}")
                    a2a_out_b = dram.tile([N_CORES * 128, TPB], F32, name=f"a2a_out_{b}")
                    for j in range(N_CORES):
                        nc.sync.dma_start(
                            a2a_in_b[j * 128:(j + 1) * 128, :],
                            concatT[:, b * S + j * TPB: b * S + (j + 1) * TPB])
                    nc.gpsimd.collective_compute(
                        "AllToAll", mybir.AluOpType.bypass,
                        replica_groups=[list(range(N_CORES))],
                        ins=[a2a_in_b.opt()], outs=[a2a_out_b.opt()])
                    a2a_out.append(a2a_out_b)

                # ---- phase 5: output projection for this core's 2x256 tokens ----
                if 5 not in phases:
                    nc.sync.dma_start(outT_d[0:128, 0:TPB], concatT[0:128, 0:TPB])
                for b in range(B) if 5 in phases else []:
                    cs = [sb.tile([128, TPB], F32R, name="cs", tag=f"cs{kt}", bufs=2)
                          for kt in range(8)]
                    for kt in range(8):
                        nc.sync.dma_start(cs[kt][:],
                                          a2a_out[b][kt * 128:(kt + 1) * 128, :].bitcast(F32R))
                    for eo in range(8):
                        facc = ps_attn.tile([128, TPB], F32, name="facc", tag="sc", bufs=2)
                        for kt in range(8):
                            nc.tensor.matmul(facc[:], wo_sb[kt][:, eo * 128:(eo + 1) * 128],
                                             cs[kt][:], start=(kt == 0), stop=(kt == 7))
                        osb = sb.tile([128, TPB], F32, name="osb", tag="osb", bufs=2)
                        nc.scalar.activation(osb[:], facc[:], AF.Identity,
                                             bias=bout_sb[:, eo:eo + 1])
                        nc.sync.dma_start(
                            outT_d[eo * 128:(eo + 1) * 128, b * TPB:(b + 1) * TPB], osb[:])

    nc.compile()
    _CACHE[key] = nc
    return nc


def _host_prep(x, mask, Wqkv, bqkv, Wout, bout):
    x = np.ascontiguousarray(np.asarray(x, np.float32))
    Wqkv = np.asarray(Wqkv, np.float32)
    bqkv = np.asarray(bqkv, np.float32)
    Wout = np.asarray(Wout, np.float32)
    bout = np.asarray(bout, np.float32)
    mask = np.asarray(mask)

    xT = np.ascontiguousarray(x.reshape(T, E).T)                       # [E, T]
    m = mask.reshape(B, S)
    ab = np.where(m == 0, np.float32(-30000.0), np.float32(0.0)).astype(np.float32)
    abias_sb = np.ascontiguousarray(ab.reshape(B, KT_S, 128).transpose(2, 0, 1)
                                    .reshape(128, B * KT_S))
    woutT = np.ascontiguousarray(Wout.T)                               # [e_in, e_out]
    bout_sb = np.ascontiguousarray(bout.reshape(8, 128).T)
    ident = np.vstack([np.eye(64, dtype=np.float32)] * 2)
    chain = np.zeros((1, 128), np.float32)

    in_maps = []
    for c in range(N_CORES):
        hs = [HPC * c + i for i in range(HPC)]
        rows = []
        for tix in range(3):  # q, k, v
            for h in hs:
                rows.append(Wqkv[tix * E + h * D: tix * E + (h + 1) * D])
        Wc = np.concatenate(rows, axis=0)                              # [384, 1024]
        wqkvT_c = np.ascontiguousarray(Wc.T)                           # [1024, 384]
        brows = []
        for tix in range(3):
            for h in hs:
                brows.append(bqkv[tix * E + h * D: tix * E + (h + 1) * D])
        bq_c = np.concatenate(brows).reshape(3, 128).T                 # [128, 3]
        in_maps.append({
            "xT": xT, "wqkvT": wqkvT_c, "bqkv_sb": np.ascontiguousarray(bq_c),
            "woutT": woutT, "bout_sb": bout_sb, "abias_sb": abias_sb,
            "ident": ident, "chain": chain,
        })
    return in_maps


def _assemble(results):
    out = np.empty((B, S, E), np.float32)
    for c in range(N_CORES):
        outT_c = results[c]["outT"]                                    # [E, 2*TPB]
        for b in range(B):
            out[b, c * TPB:(c + 1) * TPB, :] = outT_c[:, b * TPB:(b + 1) * TPB].T
    return out


def kernel(x, mask, Wqkv, bqkv, Wout, bout):
    nc = _build()
    in_maps = _host_prep(x, mask, Wqkv, bqkv, Wout, bout)
    res = bass_utils.run_bass_kernel_spmd(nc, in_maps, core_ids=list(range(N_CORES)))
    return _assemble(res.results)



# revision 11
# speedup vs baseline: 2.6788x; 2.6788x over previous
"""BERT multi-head attention forward on 8 Trainium2 NeuronCores.

Sharding: tensor-parallel over heads (16 heads -> 2 per core) for the QKV
projection and attention; a per-batch AllToAll then redistributes the
attention outputs token-wise so each core computes the output projection for
its own 512-token slice (no AllReduce needed).

v2 (engine-balanced, all-bf16 matmuls):
  - x / Wqkv / Wout are converted to bf16 on the host; every matmul runs
    with bf16 operands (1 col/cycle on the PE, 1024-wide moving operands).
  - W_k is pre-scaled by ALPHA = 128*0.125/ln2 so the score matmul directly
    produces s' = ALPHA * (q.k); exp is then exp(ln2/128 * s') on ACT, or a
    Schraudolph bit-trick on DVE (s'+beta -> int16 -> bitcast bf16).
  - exp outputs bf16 directly (no f32r recast); attention weights feed the
    PV matmul as bf16.
  - All PSUM->SBUF moves run on the Vector engine (tensor_scalar add-bias),
    keeping the Scalar engine free for exp.
  - sum-of-exp rides row 64 of the PV output (ones column in V); the
    normalization reciprocal is one batched reciprocal_approx_fast call.
  - AllToAll payload is bf16 (half the f32 bytes).

Per-core layouts:
  xT      [E=1024, T=4096] bf16  x transposed (embed on partitions)
  wqkvT   [1024, 384] bf16       this core's Wqkv rows (qA qB kA kB vA vB), transposed
  qkvT    [384, 4096] bf16 SBUF  j rows: q(128) k(128) v(128); each 128 = headA(64)+headB(64)
  vaug    [128, 132] bf16        per (b,kt): headA V(0:64)+ones(64), headB V(66:130)+ones(130)
  scp     [128, 1024] f32 PSUM   scores for one (b,qc,kt,h): 128 keys x 1024 queries
  ex      [128, 1024] bf16       exp'd scores
  oaug    [65, 1024] f32 PSUM    rows 0-63 unnormalized attn out (d x q), row 64 sumexp
  concatT [128, 4096] bf16       this core's 2 heads' channels x all tokens (normalized)
  A2A     per batch: blocks of [128 ch, 256 tok] bf16
  outT    [1024, 512] f32        output projection result, this core's tokens
"""

import numpy as np
from concourse import bacc, tile, bass_utils, mybir

F32 = mybir.dt.float32
BF16 = mybir.dt.bfloat16
I16 = mybir.dt.int16
AF = mybir.ActivationFunctionType
ALU = mybir.AluOpType

B, S, E, H, D = 2, 2048, 1024, 16, 64
T = B * S                  # 4096 tokens
N_CORES = 8
HPC = H // N_CORES         # 2 heads per core
TC = 1024                  # t-chunk for QKV projection (bf16 moving max)
QC = 1024                  # query chunk in attention
KT_S = S // 128            # 16 key tiles per batch
TPB = T // B // N_CORES    # 256 tokens per core per batch (A2A block)

ALPHA = 128.0 * 0.125 / np.log(2.0)   # fold into W_k: s' = ALPHA * (q.k)
ACT_SCALE = float(np.log(2.0) / 128.0)  # exp(ACT_SCALE * s') == exp(0.125 * q.k)
SCHRAUD_DELTA = -45.0                 # tuning offset for the bit-trick bias

# Fraction control: tile (b,qc,kt,h) goes to DVE iff (kt*HPC+h) % SCHRAUD_MOD
# < SCHRAUD_CNT.  (0 disables the DVE path.)
SCHRAUD_CNT = 0
SCHRAUD_MOD = 3

_CACHE = {}


def _build(k_rep=1):
    key = (k_rep, SCHRAUD_CNT, SCHRAUD_MOD)
    if key in _CACHE:
        return _CACHE[key]
    nc = bacc.Bacc("TRN2", target_bir_lowering=False, debug=False, num_devices=N_CORES)

    xT = nc.dram_tensor("xT", [E, T], BF16, kind="ExternalInput").ap()
    wqkvT = nc.dram_tensor("wqkvT", [E, 3 * 128], BF16, kind="ExternalInput").ap()
    bqkv_d = nc.dram_tensor("bqkv_sb", [128, 3], F32, kind="ExternalInput").ap()
    woutT = nc.dram_tensor("woutT", [E, E], BF16, kind="ExternalInput").ap()
    bout_d = nc.dram_tensor("bout_sb", [128, 8], F32, kind="ExternalInput").ap()
    abias_d = nc.dram_tensor("abias_sb", [128, B * KT_S], F32, kind="ExternalInput").ap()
    sbias_d = nc.dram_tensor("sbias_sb", [128, B * KT_S], F32, kind="ExternalInput").ap()
    ident_d = nc.dram_tensor("ident", [128, 64], BF16, kind="ExternalInput").ap()
    chain_d = nc.dram_tensor("chain", [1, 128], F32, kind="ExternalInput").ap()

    outT_d = nc.dram_tensor("outT", [E, 2 * TPB], F32, kind="ExternalOutput").ap()
    chout_d = nc.dram_tensor("chain_out", [1, 128], F32, kind="ExternalOutput").ap()

    with tile.TileContext(nc) as tc:
        with tc.tile_pool(name="sb", bufs=1) as sb, \
             tc.tile_pool(name="ps", bufs=1, space="PSUM") as ps, \
             tc.tile_pool(name="dram", bufs=1, space="DRAM") as dram:

            # chain passthrough (timing harness hook; negligible cost)
            ch_sb = sb.tile([1, 128], F32)
            nc.sync.dma_start(ch_sb[:], chain_d[:])
            nc.vector.tensor_copy(ch_sb[:], ch_sb[:])
            nc.sync.dma_start(chout_d[:], ch_sb[:])

            # ---- constants ----
            bqkv_sb = sb.tile([128, 3], F32)
            bout_sb = sb.tile([128, 8], F32)
            abias_sb = sb.tile([128, B * KT_S], F32)
            sbias_sb = sb.tile([128, B * KT_S], F32)
            ident_sb = sb.tile([128, 64], BF16)
            ones_sb = sb.tile([1, 64], BF16)
            nc.sync.dma_start(bqkv_sb[:], bqkv_d[:])
            nc.sync.dma_start(bout_sb[:], bout_d[:])
            nc.sync.dma_start(abias_sb[:], abias_d[:])
            nc.sync.dma_start(sbias_sb[:], sbias_d[:])
            nc.sync.dma_start(ident_sb[:], ident_d[:])
            nc.vector.memset(ones_sb[:], 1.0)

            # ---- weights ----
            wq_sb = [sb.tile([128, 3 * 128], BF16, name=f"wq_{e}") for e in range(8)]
            for e in range(8):
                nc.sync.dma_start(wq_sb[e][:], wqkvT[e * 128:(e + 1) * 128, :])
            wo_sb = [sb.tile([128, E], BF16, name=f"wo_{e}") for e in range(8)]
            for e in range(8):
                nc.sync.dma_start(wo_sb[e][:], woutT[e * 128:(e + 1) * 128, :])

            for _rep in range(k_rep):
                # ---- phase 1: QKV projection (d-major output, bf16) ----
                qkvT = [sb.tile([128, T], BF16, name=f"qkvT_{j}") for j in range(3)]
                concatT = sb.tile([128, T], BF16)
                for i in range(T // TC):
                    xt = [sb.tile([128, TC], BF16, name="xt", tag=f"xt{e}", bufs=2)
                          for e in range(8)]
                    for e in range(8):
                        nc.sync.dma_start(
                            xt[e][:], xT[e * 128:(e + 1) * 128, i * TC:(i + 1) * TC])
                    for j in range(3):
                        acc = ps.tile([128, TC], F32, name="acc", tag="scp", bufs=2)
                        for half in range(TC // 512):
                            hs_ = slice(half * 512, (half + 1) * 512)
                            for e in range(8):
                                nc.tensor.matmul(acc[:, hs_],
                                                 wq_sb[e][:, j * 128:(j + 1) * 128],
                                                 xt[e][:, hs_],
                                                 start=(e == 0), stop=(e == 7))
                        with nc.allow_low_precision(reason="bf16 qkv"):
                            nc.vector.tensor_scalar(
                                out=qkvT[j][:, i * TC:(i + 1) * TC], in0=acc[:],
                                scalar1=bqkv_sb[:, j:j + 1], scalar2=None, op0=ALU.add)

                # ---- phase 2: V natural layout via PE transposes, + ones cols ----
                # vaug[b][kt]: [128 keys, 132] = A(0:64) onesA(64) | B(66:130) onesB(130)
                vaug = {}
                for b in range(B):
                    for kt in range(KT_S):
                        v = sb.tile([128, 132], BF16, name=f"vaug_{b}_{kt}")
                        for h in range(HPC):
                            tp = ps.tile([128, 64], BF16, name="tp", tag="scp", bufs=2)
                            nc.tensor.transpose(
                                tp[:],
                                qkvT[2][64 * h:64 * h + 64,
                                        b * S + kt * 128: b * S + (kt + 1) * 128],
                                ident_sb[64 * h:64 * h + 64, 0:64])
                            with nc.allow_low_precision(reason="bf16 v"):
                                nc.vector.tensor_copy(v[:, 66 * h:66 * h + 64], tp[:])
                        nc.vector.memset(v[:, 64:65], 1.0)
                        nc.vector.memset(v[:, 130:131], 1.0)
                        vaug[b, kt] = v

                # ---- phase 3: attention ----
                def attention(b):
                    for qc in range(S // QC):
                        q0 = b * S + qc * QC
                        oaug = {h: ps.tile([65, QC], F32, name=f"oaug{h}",
                                           tag=f"oaug{h}")
                                for h in range(HPC)}
                        for kt in range(KT_S):
                            k0 = b * S + kt * 128
                            exs = {}
                            for h in range(HPC):
                                scp = ps.tile([128, QC], F32, name="sc",
                                              tag="scp", bufs=2)
                                for half in range(QC // 512):
                                    hs_ = slice(half * 512, (half + 1) * 512)
                                    nc.tensor.matmul(
                                        scp[:, hs_],
                                        qkvT[1][64 * h:64 * h + 64, k0:k0 + 128],
                                        qkvT[0][64 * h:64 * h + 64,
                                                q0 + half * 512:q0 + (half + 1) * 512],
                                        start=True, stop=True)
                                ex = sb.tile([128, QC], I16, name="ex",
                                             tag="ex", bufs=3)
                                col = b * KT_S + kt
                                if (kt * HPC + h) % SCHRAUD_MOD < SCHRAUD_CNT:
                                    with nc.allow_low_precision(reason="schraudolph"):
                                        nc.vector.tensor_scalar(
                                            out=ex[:], in0=scp[:],
                                            scalar1=sbias_sb[:, col:col + 1],
                                            scalar2=0.0, op0=ALU.add, op1=ALU.max)
                                else:
                                    with nc.allow_low_precision(reason="bf16 attn w"):
                                        nc.scalar.activation(
                                            ex[:].bitcast(BF16), scp[:], AF.Exp,
                                            scale=ACT_SCALE,
                                            bias=abias_sb[:, col:col + 1])
                                exs[h] = ex
                            for h in range(HPC):
                                for half in range(QC // 512):
                                    hs_ = slice(half * 512, (half + 1) * 512)
                                    nc.tensor.matmul(
                                        oaug[h][:, hs_],
                                        vaug[b, kt][:, 66 * h:66 * h + 65],
                                        exs[h][:, hs_].bitcast(BF16),
                                        start=(kt == 0), stop=(kt == KT_S - 1))
                        # tail: normalize into concatT (sumexp rows side by side
                        # on partition 0 -- engine partition base must be 0/32/64)
                        sums = sb.tile([1, HPC * QC], F32, name="sums", tag="sums", bufs=2)
                        for h in range(HPC):
                            nc.vector.tensor_copy(sums[:, h * QC:(h + 1) * QC],
                                                  oaug[h][64:65, :])
                        rec = sb.tile([1, HPC * QC], F32, name="rec", tag="rec", bufs=2)
                        nc.vector.reciprocal_approx_fast(rec[:], sums[:])
                        recr = sb.tile([1, HPC * QC], BF16, name="recr",
                                       tag="recr", bufs=2)
                        with nc.allow_low_precision(reason="bf16 recip"):
                            nc.vector.tensor_copy(recr[:], rec[:])
                        for h in range(HPC):
                            rep = ps.tile([64, QC], F32, name="rep", tag="scp", bufs=2)
                            for half in range(QC // 512):
                                hs_ = slice(half * 512, (half + 1) * 512)
                                nc.tensor.matmul(
                                    rep[:, hs_], ones_sb[:],
                                    recr[:, h * QC + half * 512:
                                         h * QC + (half + 1) * 512],
                                    start=True, stop=True)
                            reps = sb.tile([64, QC], BF16, name="reps",
                                           tag="reps", bufs=2)
                            with nc.allow_low_precision(reason="bf16 recip"):
                                nc.vector.tensor_copy(reps[:], rep[:])
                            with nc.allow_low_precision(reason="bf16 concat"):
                                nc.vector.tensor_mul(
                                    concatT[64 * h:64 * h + 64, q0:q0 + QC],
                                    oaug[h][0:64, :], reps[:])

                # ---- phase 4: per-batch AllToAll (blocks of [128 ch, 256 tok]) ----
                a2a_out = {}
                def a2a(b):
                    a2a_in_b = dram.tile([N_CORES * 128, TPB], BF16, name=f"a2a_in_{b}")
                    a2a_out_b = dram.tile([N_CORES * 128, TPB], BF16, name=f"a2a_out_{b}")
                    for j in range(N_CORES):
                        nc.sync.dma_start(
                            a2a_in_b[j * 128:(j + 1) * 128, :],
                            concatT[:, b * S + j * TPB: b * S + (j + 1) * TPB])
                    nc.gpsimd.collective_compute(
                        "AllToAll", mybir.AluOpType.bypass,
                        replica_groups=[list(range(N_CORES))],
                        ins=[a2a_in_b.opt()], outs=[a2a_out_b.opt()])
                    a2a_out[b] = a2a_out_b

                # ---- phase 5: output projection for this core's 256 tokens of b ----
                def outproj(b):
                    cs = [sb.tile([128, TPB], BF16, name="cs", tag=f"cs{kt}", bufs=2)
                          for kt in range(8)]
                    for kt in range(8):
                        nc.sync.dma_start(cs[kt][:],
                                          a2a_out[b][kt * 128:(kt + 1) * 128, :])
                    for eo in range(8):
                        facc = ps.tile([128, TPB], F32, name="facc", tag="scp", bufs=2)
                        for kt in range(8):
                            nc.tensor.matmul(facc[:], wo_sb[kt][:, eo * 128:(eo + 1) * 128],
                                             cs[kt][:], start=(kt == 0), stop=(kt == 7))
                        osb = sb.tile([128, TPB], F32, name="osb", tag="osb", bufs=2)
                        nc.vector.tensor_scalar(
                            out=osb[:], in0=facc[:],
                            scalar1=bout_sb[:, eo:eo + 1], scalar2=None, op0=ALU.add)
                        nc.sync.dma_start(
                            outT_d[eo * 128:(eo + 1) * 128, b * TPB:(b + 1) * TPB], osb[:])

                attention(0)
                a2a(0)
                attention(1)
                outproj(0)
                a2a(1)
                outproj(1)

    nc.compile()
    _CACHE[key] = nc
    return nc


def _host_prep(x, mask, Wqkv, bqkv, Wout, bout):
    import ml_dtypes
    bf16 = ml_dtypes.bfloat16
    x = np.ascontiguousarray(np.asarray(x, np.float32))
    Wqkv = np.asarray(Wqkv, np.float32)
    bqkv = np.asarray(bqkv, np.float32)
    Wout = np.asarray(Wout, np.float32)
    bout = np.asarray(bout, np.float32)
    mask = np.asarray(mask)

    xT = np.ascontiguousarray(x.reshape(T, E).T.astype(bf16))          # [E, T]
    m = mask.reshape(B, S)
    ab = np.where(m == 0, np.float32(-30000.0), np.float32(0.0)).astype(np.float32)
    abias_sb = np.ascontiguousarray(ab.reshape(B, KT_S, 128).transpose(2, 0, 1)
                                    .reshape(128, B * KT_S))
    # Schraudolph bias: beta for live keys, very negative for masked keys
    beta = np.float32(127.0 * 128.0 + SCHRAUD_DELTA)
    sb_b = np.where(m == 0, np.float32(-1e7), beta).astype(np.float32)
    sbias_sb = np.ascontiguousarray(sb_b.reshape(B, KT_S, 128).transpose(2, 0, 1)
                                    .reshape(128, B * KT_S))
    woutT = np.ascontiguousarray(Wout.T.astype(bf16))                  # [e_in, e_out]
    bout_sb = np.ascontiguousarray(bout.reshape(8, 128).T)
    ident = np.vstack([np.eye(64, dtype=np.float32)] * 2).astype(bf16)
    chain = np.zeros((1, 128), np.float32)

    in_maps = []
    for c in range(N_CORES):
        hs = [HPC * c + i for i in range(HPC)]
        rows = []
        for tix in range(3):  # q, k, v
            scale = ALPHA if tix == 1 else 1.0
            for h in hs:
                rows.append(Wqkv[tix * E + h * D: tix * E + (h + 1) * D] * scale)
        Wc = np.concatenate(rows, axis=0)                              # [384, 1024]
        wqkvT_c = np.ascontiguousarray(Wc.T.astype(bf16))              # [1024, 384]
        brows = []
        for tix in range(3):
            scale = ALPHA if tix == 1 else 1.0
            for h in hs:
                brows.append(bqkv[tix * E + h * D: tix * E + (h + 1) * D] * scale)
        bq_c = np.concatenate(brows).reshape(3, 128).T                 # [128, 3]
        in_maps.append({
            "xT": xT, "wqkvT": wqkvT_c,
            "bqkv_sb": np.ascontiguousarray(bq_c.astype(np.float32)),
            "woutT": woutT, "bout_sb": bout_sb, "abias_sb": abias_sb,
            "sbias_sb": sbias_sb, "ident": ident, "chain": chain,
        })
    return in_maps


def _assemble(results):
    out = np.empty((B, S, E), np.float32)
    for c in range(N_CORES):
        outT_c = results[c]["outT"]                                    # [E, 2*TPB]
        for b in range(B):
            out[b, c * TPB:(c + 1) * TPB, :] = outT_c[:, b * TPB:(b + 1) * TPB].T
    return out


def kernel(x, mask, Wqkv, bqkv, Wout, bout):
    nc = _build()
    in_maps = _host_prep(x, mask, Wqkv, bqkv, Wout, bout)
    res = bass_utils.run_bass_kernel_spmd(nc, in_maps, core_ids=list(range(N_CORES)))
    return _assemble(res.results)
